# revision 53
# baseline (speedup 1.0000x reference)
"""HeightmapNormalsLoss TRN2 kernel (v3).

Data-parallel over 8 NeuronCores: 4 image-pairs per core, software-pipelined
over the 8 images (stage1 = DMA+Sobel+extract, stage2a = q/r on DVE,
stage2b = sqrts on ACT + normalize mults on DVE, then per-pair |diff| and a
TensorEngine ones-vector reduction). Per image:

  Sobel gx'/gy' via TensorE band matmuls (weights pre-scaled by sqrt63)
  q   = gx'^2 + gy'^2 + 1          (custom DVE op SQSUM1_HNL, 2x uops)
  r   = 1/q                        (DVE RECIPROCAL_APPROX_FAST, 1x)
  i'  = sqrt(16/63 * r)            (ACT Sqrt; nx = gx'*i' = 4gx/sqrt(q))
  nz  = sqrt(64/63 * r - 1/63)     (ACT Sqrt, direct -- no multiply needed)
  nx  = gx'*i', ny = gy'*i'        (DVE TT, stock 2x mode)
per pair:
  dd  = |n_gen - n_tgt|            (custom DVE op ABSDIFF2_HNL, 2x uops)
  acc_ps[1,512] += ones[0:M].T @ dd[0:M]   (TensorE, accumulating PSUM)

The two custom DVE ops carry hand-authored 2x_1P uop programs (selected via
perf_max=1 in the instruction encoding); the per-NEFF DVE table is generated
at compile time by concourse's dve_table_gen. Pipeline-fill images 0-1 load
f32 via HWDGE (avoids the SWDGE IRAM warmup) and cast on the then-idle ACT.

Per-core output: [1, 512] f32 column sums; host reduces and divides.
"""
import sys

sys.path.insert(0, "/opt/trn_rl_repo")

import numpy as np
import ml_dtypes

H = W = 512
N_CORES = 8
PAIRS_PER_CORE = 4
TOTAL_B = 32
SQ63 = float(np.sqrt(63.0))
import os
PM_SQSUM = int(os.environ.get('PM_SQSUM', '1'))
PM_ABSDIFF = int(os.environ.get('PM_ABSDIFF', '1'))

# (out_row_start, M, in_row_start, K, variant_idx)
ROW_TILES = [
    (0, 127, 0, 128, 0),
    (127, 126, 126, 128, 1),
    (253, 126, 252, 128, 1),
    (379, 126, 378, 128, 1),
    (505, 7, 504, 8, 2),
]
NT = len(ROW_TILES)  # 5
N_ACC_COLS = PAIRS_PER_CORE * NT  # 20


def _build_bands_np():
    """[128, 12*128] f32: blocks (band*3 + variant), bands sv, -sv, dv, 2dv.
    All scaled by sqrt(63) so that q = gx'^2 + gy'^2 + 1 = 63*s + 1."""
    mats = {}
    for v, (K, M) in enumerate([(128, 127), (128, 126), (8, 7)]):
        sv = np.zeros((128, 128), np.float32)
        dv = np.zeros((128, 128), np.float32)
        if v == 0:  # first: m=0 clamps row -1 -> 0
            sv[0, 0], sv[1, 0] = 3.0, 1.0
            dv[0, 0], dv[1, 0] = 1.0, -1.0
            for m in range(1, M):
                sv[m - 1, m], sv[m, m], sv[m + 1, m] = 1.0, 2.0, 1.0
                dv[m - 1, m], dv[m + 1, m] = 1.0, -1.0
        elif v == 1:  # mid
            for m in range(M):
                sv[m, m], sv[m + 1, m], sv[m + 2, m] = 1.0, 2.0, 1.0
                dv[m, m], dv[m + 2, m] = 1.0, -1.0
        else:  # last: m=M-1 (global 511) clamps row 512 -> 511
            for m in range(M - 1):
                sv[m, m], sv[m + 1, m], sv[m + 2, m] = 1.0, 2.0, 1.0
                dv[m, m], dv[m + 2, m] = 1.0, -1.0
            m = M - 1
            sv[m, m], sv[m + 1, m] = 1.0, 3.0
            dv[m, m], dv[m + 1, m] = 1.0, -1.0
        mats[(0, v)] = sv
        mats[(1, v)] = -sv
        mats[(2, v)] = dv
        mats[(3, v)] = 2.0 * dv
    w = np.zeros((128, 12 * 128), np.float32)
    for b in range(4):
        for v in range(3):
            w[:, (b * 3 + v) * 128 : (b * 3 + v + 1) * 128] = SQ63 * mats[(b, v)]
    return w.astype(ml_dtypes.bfloat16)


def _sqsum_uops_2x():
    """2x_1P program for SQSUM: per cycle compute q = a^2 + b^2 + 1 for both
    the lo and hi packed bf16 elements. 8 ALU blocks, chains 0-4."""
    from concourse.dve_uop import (
        UopConfig, InpSel, AluInp, DelayInp, OutSel, OutPath, Trigger, ENABLE,
    )
    from concourse.dve_uop import AluOp as UAlu

    u = UopConfig()
    u.enable_input(InpSel.SRC_0, 1)     # chain0 = a_lo
    u.enable_input(InpSel.SRC_1, 2)     # chain1 = b_lo
    u.enable_input(InpSel.SRC_0_HI, 3)  # chain2 = a_hi
    u.enable_input(InpSel.SRC_1_HI, 4)  # chain3 = b_hi
    u.enable_input(InpSel.ONE_F32, 5)   # chain4 = 1.0
    u.require_inp0 = ENABLE
    u.require_inp1 = ENABLE
    u.trigger = (Trigger.SRC_TENSOR_DONE, Trigger.NONE, Trigger.NONE)
    dp = u.datapath_config
    dp[0].enable_alu(UAlu.MULTIPLY, AluInp.PREV_DELAY_0, AluInp.PREV_DELAY_0)
    dp[0].pass_through_delay(1, 2, 3, 4)
    dp[1].enable_alu(UAlu.MULTIPLY, AluInp.PREV_DELAY_1, AluInp.PREV_DELAY_1)
    dp[1].enable_delay_from_src(DelayInp.PREV_ALU_OUT, 0)  # a_lo^2
    dp[1].pass_through_delay(2, 3, 4)
    dp[2].enable_alu(UAlu.ADD, AluInp.PREV_ALU_OUT, AluInp.PREV_DELAY_0)
    dp[2].pass_through_delay(2, 3, 4)
    dp[3].enable_alu(UAlu.ADD, AluInp.PREV_ALU_OUT, AluInp.PREV_DELAY_4)
    dp[3].pass_through_delay(2, 3, 4)
    dp[4].enable_alu(UAlu.MULTIPLY, AluInp.PREV_DELAY_2, AluInp.PREV_DELAY_2)
    dp[4].enable_delay_from_src(DelayInp.PREV_ALU_OUT, 0)  # q_lo
    dp[4].pass_through_delay(3, 4)
    dp[5].enable_alu(UAlu.MULTIPLY, AluInp.PREV_DELAY_3, AluInp.PREV_DELAY_3)
    dp[5].enable_delay_from_src(DelayInp.PREV_ALU_OUT, 1)  # a_hi^2
    dp[5].pass_through_delay(0, 4)
    dp[6].enable_alu(UAlu.ADD, AluInp.PREV_ALU_OUT, AluInp.PREV_DELAY_1)
    dp[6].pass_through_delay(0, 4)
    dp[7].enable_alu(UAlu.ADD, AluInp.PREV_ALU_OUT, AluInp.PREV_DELAY_4)
    dp[7].pass_through_delay(0)
    u.enable_output(OutSel.DELAY_0, OutPath.WR0_LO)  # q_lo
    u.enable_output(OutSel.ALU_OUT, OutPath.WR0_HI)  # q_hi
    return [u]


def _absdiff2_uops_2x():
    """2x_1P program for plain ABSDIFF (no accumulator): per cycle |a-b| for
    lo and hi packed bf16 elements. hi marches down the ALU bypass chain to
    block 7 (ALU_OUT); lo rides delay chain 0."""
    from concourse.dve_uop import (
        UopConfig, InpSel, AluInp, DelayInp, OutSel, OutPath, Trigger, ENABLE,
    )
    from concourse.dve_uop import AluOp as UAlu

    u = UopConfig()
    u.enable_input(InpSel.SRC_0, 1)     # chain0 = a_lo
    u.enable_input(InpSel.SRC_1, 2)     # chain1 = b_lo
    u.enable_input(InpSel.SRC_0_HI, 3)  # chain2 = a_hi
    u.enable_input(InpSel.SRC_1_HI, 4)  # chain3 = b_hi
    u.require_inp0 = ENABLE
    u.require_inp1 = ENABLE
    u.trigger = (Trigger.SRC_TENSOR_DONE, Trigger.NONE, Trigger.NONE)
    dp = u.datapath_config
    dp[0].enable_alu(UAlu.ABSOLUTE_DIFF, AluInp.PREV_DELAY_0, AluInp.PREV_DELAY_1)
    dp[0].pass_through_delay(2, 3)
    dp[1].enable_alu(UAlu.ABSOLUTE_DIFF, AluInp.PREV_DELAY_2, AluInp.PREV_DELAY_3)
    dp[1].enable_delay_from_src(DelayInp.PREV_ALU_OUT, 0)  # |d|_lo
    for bi in range(2, 8):
        dp[bi].pass_through_alu()
        dp[bi].pass_through_delay(0)
    u.enable_output(OutSel.DELAY_0, OutPath.WR0_LO)  # |d|_lo
    u.enable_output(OutSel.ALU_OUT, OutPath.WR0_HI)  # |d|_hi
    return [u]


def _absdiff_uops_2x():
    """2x_1P program for ABSDIFF_ACC: per cycle |a-b| for lo and hi packed
    elements, elementwise outs on WR0_LO/WR0_HI, accumulator += lo+hi.
    Two uops: [0] = 1-cycle accumulator seed (Zero), [1] = steady state."""
    from concourse.dve_uop import (
        UopConfig, InpSel, AluInp, DelayInp, OutSel, OutPath, Trigger, ENABLE,
    )
    from concourse.dve_uop import AluOp as UAlu

    def lanes(u):
        u.enable_input(InpSel.SRC_0, 1)     # chain0 = a_lo
        u.enable_input(InpSel.SRC_1, 2)     # chain1 = b_lo
        u.enable_input(InpSel.SRC_0_HI, 3)  # chain2 = a_hi
        u.enable_input(InpSel.SRC_1_HI, 4)  # chain3 = b_hi
        u.enable_input(InpSel.ZERO, 5)      # chain4 = 0.0 (accum seed)

    u0 = UopConfig()
    lanes(u0)
    u0.repeat_count = 1
    u0.trigger = (Trigger.COUNT, Trigger.NONE, Trigger.NONE)
    u0.next_uop = (1, 0, 0)
    u0.accum_enabled = ENABLE
    dp = u0.datapath_config
    # carry the ZERO const (chain4) to block 3 and seed the accumulator there
    # (steady state's accumulate-ADD reads CURR_ALU_OUT at block 3).
    for bi in range(3):
        dp[bi].pass_through_alu()
        dp[bi].pass_through_delay(4)
    dp[3].enable_alu(UAlu.BYPASS, AluInp.PREV_DELAY_4, AluInp.PREV_DELAY_4)
    dp[3].alu_out_a_enable = ENABLE
    for bi in range(4, 8):
        dp[bi].pass_through_alu()
        dp[bi].alu_out_a_enable = ENABLE

    u1 = UopConfig()
    lanes(u1)
    u1.require_inp0 = ENABLE
    u1.require_inp1 = ENABLE
    u1.trigger = (Trigger.SRC_TENSOR_DONE, Trigger.NONE, Trigger.NONE)
    u1.accum_enabled = ENABLE
    dp = u1.datapath_config
    dp[0].enable_alu(UAlu.ABSOLUTE_DIFF, AluInp.PREV_DELAY_0, AluInp.PREV_DELAY_1)
    dp[0].pass_through_delay(2, 3)
    dp[1].enable_alu(UAlu.ABSOLUTE_DIFF, AluInp.PREV_DELAY_2, AluInp.PREV_DELAY_3)
    dp[1].enable_delay_from_src(DelayInp.PREV_ALU_OUT, 0)  # |d|_lo
    dp[2].enable_alu(UAlu.ADD, AluInp.PREV_ALU_OUT, AluInp.PREV_DELAY_0)
    dp[2].enable_delay_from_src(DelayInp.PREV_ALU_OUT, 1)  # |d|_hi
    dp[2].pass_through_delay(0)
    dp[3].enable_alu(UAlu.ADD, AluInp.CURR_ALU_OUT, AluInp.PREV_ALU_OUT)
    dp[3].alu_out_a_enable = ENABLE
    dp[3].pass_through_delay(0, 1)
    for bi in range(4, 8):
        dp[bi].pass_through_alu()
        dp[bi].alu_out_a_enable = ENABLE
        dp[bi].pass_through_delay(0, 1)
    u1.enable_output(OutSel.DELAY_0, OutPath.WR0_LO)  # |d|_lo
    u1.enable_output(OutSel.DELAY_1, OutPath.WR0_HI)  # |d|_hi
    return [u0, u1]


def _register_custom_ops():
    """Register the two fused DVE ops into concourse.dve_ops at runtime,
    with hand-authored 2x_1P perf-mode programs."""
    from concourse import dve_ops
    from concourse.dve_spec import (
        Spec, Src0, Src1, One, Zero, Bin, AluOp, sq, lower, _has_src1,
    )
    from concourse.dve_uop import DveOpSpec
    from operator import add as _add
    from dataclasses import dataclass, field as _field

    @dataclass(frozen=True)
    class DveOp2x(dve_ops.DveOp):
        uops_2x_builder: object = None

        def compile(self, ver):
            key = (self.name, ver)
            if (r := dve_ops._COMPILE_CACHE.get(key)) is not None:
                return r
            spec = DveOpSpec(
                name=self.name,
                opcode=dve_ops.get_dve_sub_opcode(self.name),
                uops=lower(self.spec, ver=ver),
                rd1_en=_has_src1(self.spec),
            )
            if ver == "v3" and self.uops_2x_builder is not None:
                spec.uops_2x = self.uops_2x_builder()
                spec.perf_max = 1
            dve_ops._COMPILE_CACHE[key] = spec
            return spec

    def mk(name, spec, subdim=False, uops_2x_builder=None):
        for o in dve_ops.OPS:
            if o.name == name:
                return o
        op = DveOp2x(
            name, spec, subdim=subdim, uops_sha={}, uops_2x_builder=uops_2x_builder
        )
        dve_ops.OPS.append(op)
        dve_ops.CUSTOM_DVE_SPECS[name] = spec
        dve_ops._SUB_OPCODE_FOR_NAME[name] = (
            dve_ops._CUSTOM_DVE_ROW_BASE + len(dve_ops.OPS) - 1
        )
        return op

    sqsum = mk(
        "SQSUM1_HNL",
        Spec(
            body=sq(Src0) + sq(Src1) + One,
            reference=lambda in0, in1, s0, s1, imm2: (
                in0.astype(np.float32) ** 2 + in1.astype(np.float32) ** 2 + 1.0
            ),
        ),
        uops_2x_builder=_sqsum_uops_2x,
    )

    def _absdiff_ref(in0, in1, s0, s1, imm2):
        b = np.abs(in0.astype(np.float32) - in1.astype(np.float32)).astype(np.float32)
        return b, b.reshape(b.shape[0], -1).sum(axis=-1, keepdims=True)

    absdiff = mk(
        "ABSDIFF2_HNL",
        Spec(
            body=Bin(AluOp.ABSOLUTE_DIFF, Src0, Src1),
            reference=lambda in0, in1, s0, s1, imm2: np.abs(
                in0.astype(np.float32) - in1.astype(np.float32)
            ).astype(np.float32),
        ),
        uops_2x_builder=_absdiff2_uops_2x,
    )
    return sqsum, absdiff


def _emit_custom_dve(vec, op, *, out, in0, in1=None, s0=0.0, s1=0.0, imm2=0.0,
                     accum_out=None, perf_max=0):
    """Mirror of bass.Vector._custom_dve that also encodes perf_max (byte-36
    [7:6]) so the engine may select the 2x_1P uop slot."""
    from concourse import bass_isa
    from concourse.bass import dve_ver_for
    from concourse.dve_ops import get_dve_sub_opcode

    bass = vec.bass
    if op.name not in bass.m.ant_custom_dve_ops:
        bass.m.ant_custom_dve_ops = sorted({*bass.m.ant_custom_dve_ops, op.name})
    ver = dve_ver_for(bass.trn_type)
    compiled = op.compile(ver)
    opt = not op.subdim
    in1_elementwise = len(in1.shape) > 2 if in1 is not None else False
    shape = (
        bass_isa.CustomDveShape.STT
        if in1_elementwise
        else bass_isa.CustomDveShape.TTSS
    )
    isa_opcode = bass.isa.Opcode[
        f"NEURON_ISA_TPB_OPCODE_CUSTOM_DVE_ANT_{shape.slot()}"
    ].value
    ins = [vec.lower_ap(in0, for_isa=True, opt=opt)]
    if in1 is not None:
        ins.append(vec.lower_ap(in1, for_isa=True, opt=opt))
    ins += [vec.lower_ap_or_imm(s0, for_isa=True), vec.lower_ap_or_imm(s1, for_isa=True)]
    outs = [vec.lower_ap(out, for_isa=True, opt=opt)]
    if accum_out is not None:
        outs.append(vec.lower_ap(accum_out, for_isa=True))
    return vec.add_instruction(
        bass_isa.InstCustomDveAnt(
            name=bass.get_next_instruction_name(),
            op_name=op.name,
            rd1_en=compiled.rd1_en,
            subdim=0x02 if op.subdim else 0,
            imm2=imm2,
            shape=shape,
            row=get_dve_sub_opcode(op.name),
            isa_opcode=isa_opcode,
            ins=ins,
            outs=outs,
            perf_max=perf_max,
        )
    )


def _kernel_body(tc, gen_d, tgt_d, w_d, acc_d, sqsum_op, absdiff_op):
    from contextlib import ExitStack
    from concourse import mybir
    from concourse.dve_ops import RECIPROCAL_APPROX_FAST, RECIP_APPROX_FAST_CONSTS

    nc = tc.nc
    AF = mybir.ActivationFunctionType
    OP = mybir.AluOpType
    f32 = mybir.dt.float32
    bf16 = mybir.dt.bfloat16

    with ExitStack() as ctx:
        persist = ctx.enter_context(tc.tile_pool(name="persist", bufs=1))
        xp_pool = ctx.enter_context(tc.tile_pool(name="xp", bufs=4))
        ps_pool = ctx.enter_context(tc.tile_pool(name="ps", bufs=3, space="PSUM"))
        acc_ps_pool = ctx.enter_context(tc.tile_pool(name="accps", bufs=1, space="PSUM"))
        gxy_pool = ctx.enter_context(tc.tile_pool(name="gxy", bufs=3))
        q_pool = ctx.enter_context(tc.tile_pool(name="q", bufs=2))
        r_pool = ctx.enter_context(tc.tile_pool(name="r", bufs=3))
        i_pool = ctx.enter_context(tc.tile_pool(name="i", bufs=2))
        n_pool = ctx.enter_context(tc.tile_pool(name="n", bufs=3))
        dd_pool = ctx.enter_context(tc.tile_pool(name="dd", bufs=2))
        xpf_pool = ctx.enter_context(tc.tile_pool(name="xpf", bufs=2))

        wt = persist.tile([128, 12 * 128], bf16)
        nc.sync.dma_start(wt[:], w_d[:])
        ones_t = persist.tile([128, 1], bf16)
        nc.vector.memset(ones_t[:], 1.0)
        bias0 = persist.tile([128, 1], f32)
        nc.vector.memset(bias0[:], 0.0)
        bias_nz = persist.tile([128, 1], f32)
        nc.vector.memset(bias_nz[:], -1.0 / 63.0)
        acc_ps = acc_ps_pool.tile([1, 512], f32)

        def w_sl(band, variant, K, M):
            blk = (band * 3 + variant) * 128
            return wt[0:K, blk : blk + M]

        # Software-pipelined over the 8 images (pair p -> images 2p, 2p+1).
        # stage1(k):  DMA + pads + matmuls + PSUM extraction  (TE + ACT)
        # stage2a(k): q = SQSUM, r = recip                    (DVE)
        # stage2b(k): i'/nz sqrts + nx,ny mults               (ACT + DVE)
        # absdiff(j): after stage2b of image 2j+1             (DVE)
        # Emission order staggers stages so every engine queue always holds
        # independent work (avoids ACT/DVE head-of-line ping-pong).
        N_IMG = 2 * PAIRS_PER_CORE
        state = [dict() for _ in range(N_IMG)]

        def src_of(k):
            return (gen_d if k % 2 == 0 else tgt_d), k // 2

        def stage1_dma(k):
            src, pair = src_of(k)
            xp = xp_pool.tile([128, NT, W + 2], bf16, tag="xp")
            state[k]["xp"] = xp
            if k < 2:
                # pipeline fill: HWDGE (no SWDGE IRAM warmup) f32 loads spread
                # across idle engines' queues + per-rt ACT casts so the first
                # matmuls can start as soon as rt0 lands.
                xpf = xpf_pool.tile([128, NT, W], f32, tag="xpf")
                qeng = [nc.sync, nc.scalar]
                for rt, (r0, M, i0, K, v) in enumerate(ROW_TILES):
                    qeng[(rt + k) % len(qeng)].dma_start(
                        xpf[0:K, rt, :], src[pair, i0 : i0 + K, :]
                    )
                for rt, (r0, M, i0, K, v) in enumerate(ROW_TILES):
                    nc.scalar.copy(xp[:, rt, 1 : W + 1], xpf[:, rt, :])
                    nc.vector.tensor_copy(xp[:, rt, 0:1], xp[:, rt, 1:2])
                    nc.vector.tensor_copy(
                        xp[:, rt, W + 1 : W + 2], xp[:, rt, W : W + 1]
                    )
            else:
                for rt, (r0, M, i0, K, v) in enumerate(ROW_TILES):
                    nc.gpsimd.dma_start(
                        xp[0:K, rt, 1 : W + 1], src[pair, i0 : i0 + K, :]
                    )
                nc.vector.tensor_copy(xp[:, :, 0:1], xp[:, :, 1:2])
                nc.vector.tensor_copy(xp[:, :, W + 1 : W + 2], xp[:, :, W : W + 1])

        def stage1_mm(k):
            gxy = gxy_pool.tile([128, NT, 2 * W], bf16, tag="gxy")
            for rt, (r0, M, i0, K, v) in enumerate(ROW_TILES):
                xr = state[k]["xp"][:, rt, :]
                pgx = ps_pool.tile([128, W], f32, tag="pgx")
                pgy = ps_pool.tile([128, W], f32, tag="pgy")
                dv = w_sl(2, v, K, M)
                nc.tensor.matmul(
                    pgx[0:M, :], w_sl(0, v, K, M), xr[0:K, 0:W], start=True,
                    stop=False,
                )
                nc.tensor.matmul(
                    pgx[0:M, :], w_sl(1, v, K, M), xr[0:K, 2 : W + 2],
                    start=False, stop=True,
                )
                nc.tensor.matmul(
                    pgy[0:M, :], dv, xr[0:K, 0:W], start=True, stop=False
                )
                nc.tensor.matmul(
                    pgy[0:M, :], dv, xr[0:K, 2 : W + 2], start=False, stop=False
                )
                nc.tensor.matmul(
                    pgy[0:M, :], w_sl(3, v, K, M), xr[0:K, 1 : W + 1],
                    start=False, stop=True,
                )
                nc.scalar.copy(gxy[0:M, rt, 0:W], pgx[0:M, :])
                if k < 4:
                    # early images: DVE has slack while ACT is the gate
                    nc.vector.tensor_copy(gxy[0:M, rt, W : 2 * W], pgy[0:M, :])
                else:
                    nc.scalar.copy(gxy[0:M, rt, W : 2 * W], pgy[0:M, :])
            state[k]["gxy"] = gxy

        def stage2a(k):
            gxy = state[k]["gxy"]
            q = q_pool.tile([128, NT, W], bf16, tag="q")
            _emit_custom_dve(
                nc.vector, sqsum_op, out=q[:, :, :],
                in0=gxy[:, :, 0:W], in1=gxy[:, :, W : 2 * W], perf_max=PM_SQSUM,
            )
            r = r_pool.tile([128, NT, W], bf16, tag="r")
            nc.vector._custom_dve(
                RECIPROCAL_APPROX_FAST, out=r[:, :, :], in0=q[:, :, :],
                **RECIP_APPROX_FAST_CONSTS,
            )
            state[k]["r"] = r

        def stage2b(k):
            gxy, r = state[k]["gxy"], state[k]["r"]
            n = n_pool.tile([128, NT, 3 * W], bf16, tag="n")
            ii = i_pool.tile([128, NT, W], bf16, tag="ii")
            nc.scalar.activation(
                ii[:, :, :], r[:, :, :], AF.Sqrt, bias=bias0[:, :], scale=16.0 / 63.0
            )
            nc.scalar.activation(
                n[:, :, 2 * W : 3 * W], r[:, :, :], AF.Sqrt,
                bias=bias_nz[:, :], scale=64.0 / 63.0,
            )
            nc.vector.tensor_tensor(n[:, :, 0:W], gxy[:, :, 0:W], ii[:, :, :], OP.mult)
            nc.vector.tensor_tensor(
                n[:, :, W : 2 * W], gxy[:, :, W : 2 * W], ii[:, :, :], OP.mult
            )
            state[k]["n"] = n

        def absdiff(j, rts=None):
            # |n_gen - n_tgt| (junk rows beyond M are computed but never read
            # by the reduction matmuls). rts selects a row-tile subset so the
            # last pair can interleave absdiff with its reduce matmuls.
            n0, n1 = state[2 * j]["n"], state[2 * j + 1]["n"]
            dd = state[2 * j].get("dd")
            if dd is None:
                dd = dd_pool.tile([128, NT, 3 * W], bf16, tag="dd")
                state[2 * j]["dd"] = dd
            if rts is None:
                _emit_custom_dve(
                    nc.vector, absdiff_op,
                    out=dd[:, :, :], in0=n0[:, :, :], in1=n1[:, :, :],
                    perf_max=PM_ABSDIFF,
                )
            else:
                for rt in rts:
                    _emit_custom_dve(
                        nc.vector, absdiff_op,
                        out=dd[:, rt, :], in0=n0[:, rt, :], in1=n1[:, rt, :],
                        perf_max=PM_ABSDIFF,
                    )

        def reduce_mm(j, first, last, rts=None):
            # sum dd over rows/cols via ones-vector matmuls accumulating into
            # the persistent [1, 512] PSUM tile.
            dd = state[2 * j]["dd"]
            for rt in (range(NT) if rts is None else rts):
                M = ROW_TILES[rt][1]
                for c in range(3):
                    nc.tensor.matmul(
                        acc_ps[0:1, :],
                        ones_t[0:M, 0:1],
                        dd[0:M, rt, c * W : (c + 1) * W],
                        start=(first and rt == 0 and c == 0),
                        stop=(last and rt == NT - 1 and c == 2),
                    )

        JL = PAIRS_PER_CORE - 1  # last pair: interleave per row tile
        for t in range(N_IMG + 3):
            # emission order within a step: DMAs first (latency), then the
            # older images' DVE/ACT chain stages, then this step's matmuls +
            # extraction — so ACT serves i'/nz before the new extractions and
            # DVE's normalize mults don't stall behind them.
            if t < N_IMG:
                stage1_dma(t)
            if 1 <= t <= N_IMG:
                stage2a(t - 1)
            if 2 <= t <= N_IMG + 1:
                k = t - 2
                stage2b(k)
                if k % 2 == 1:
                    j = k // 2
                    if j == JL:
                        for rt in range(NT):
                            absdiff(j, rts=[rt])
                            reduce_mm(j, first=(j == 0), last=True, rts=[rt])
                    else:
                        absdiff(j)
            if 4 <= t and (t - 4) % 2 == 0 and (t - 4) // 2 < JL:
                j = (t - 4) // 2
                reduce_mm(j, first=(j == 0), last=False)
            if t < N_IMG:
                stage1_mm(t)

        acc_sb = persist.tile([1, 512], f32)
        nc.scalar.copy(acc_sb[:], acc_ps[:])
        nc.sync.dma_start(acc_d[:], acc_sb[:])


_CACHE = {}


def _get_module():
    if "nc" not in _CACHE:
        from concourse import bacc, tile, mybir

        sqsum_op, absdiff_op = _register_custom_ops()
        nc = bacc.Bacc(
            "TRN2",
            target_bir_lowering=False,
            debug=False,
            enable_asserts=True,
            num_devices=N_CORES,
        )
        gen_d = nc.dram_tensor(
            "gen", (PAIRS_PER_CORE, H, W), mybir.dt.float32, kind="ExternalInput"
        ).ap()
        tgt_d = nc.dram_tensor(
            "tgt", (PAIRS_PER_CORE, H, W), mybir.dt.float32, kind="ExternalInput"
        ).ap()
        w_d = nc.dram_tensor(
            "w", (128, 12 * 128), mybir.dt.bfloat16, kind="ExternalInput"
        ).ap()
        acc_d = nc.dram_tensor(
            "acc", (1, 512), mybir.dt.float32, kind="ExternalOutput"
        ).ap()
        with tile.TileContext(nc) as tc:
            _kernel_body(tc, gen_d, tgt_d, w_d, acc_d, sqsum_op, absdiff_op)
        nc.compile()
        _CACHE["nc"] = nc
        _CACHE["w"] = _build_bands_np()
    return _CACHE["nc"], _CACHE["w"]


def _run(generated, target, **spmd_kwargs):
    from concourse import bass_utils

    nc, w = _get_module()
    g = np.ascontiguousarray(np.asarray(generated, np.float32).reshape(TOTAL_B, H, W))
    t = np.ascontiguousarray(np.asarray(target, np.float32).reshape(TOTAL_B, H, W))
    in_maps = [
        {
            "gen": g[c * PAIRS_PER_CORE : (c + 1) * PAIRS_PER_CORE],
            "tgt": t[c * PAIRS_PER_CORE : (c + 1) * PAIRS_PER_CORE],
            "w": w,
        }
        for c in range(N_CORES)
    ]
    return bass_utils.run_bass_kernel_spmd(
        nc, in_maps, core_ids=list(range(N_CORES)), **spmd_kwargs
    )


def kernel(generated, target):
    res = _run(generated, target)
    total = 0.0
    for r in res.results:
        total += float(np.asarray(r["acc"], np.float64).sum())
    return np.float32(total / (TOTAL_B * 3 * H * W))


# revision 55
# speedup vs baseline: 1.0054x; 1.0054x over previous
"""HeightmapNormalsLoss TRN2 kernel (v3).

Data-parallel over 8 NeuronCores: 4 image-pairs per core, software-pipelined
over the 8 images (stage1 = DMA+Sobel+extract, stage2a = q/r on DVE,
stage2b = sqrts on ACT + normalize mults on DVE, then per-pair |diff| and a
TensorEngine ones-vector reduction). Per image:

  Sobel gx'/gy' via TensorE band matmuls (weights pre-scaled by sqrt63)
  q   = gx'^2 + gy'^2 + 1          (custom DVE op SQSUM1_HNL, 2x uops)
  r   = 1/q                        (DVE RECIPROCAL_APPROX_FAST, 1x)
  i'  = sqrt(16/63 * r)            (ACT Sqrt; nx = gx'*i' = 4gx/sqrt(q))
  nz  = sqrt(64/63 * r - 1/63)     (ACT Sqrt, direct -- no multiply needed)
  nx  = gx'*i', ny = gy'*i'        (DVE TT, stock 2x mode)
per pair:
  dd  = |n_gen - n_tgt|            (custom DVE op ABSDIFF2_HNL, 2x uops)
  acc_ps[1,512] += ones[0:M].T @ dd[0:M]   (TensorE, accumulating PSUM)

The two custom DVE ops carry hand-authored 2x_1P uop programs (selected via
perf_max=1 in the instruction encoding); the per-NEFF DVE table is generated
at compile time by concourse's dve_table_gen. Pipeline-fill images 0-1 load
f32 via HWDGE (avoids the SWDGE IRAM warmup) and cast on the then-idle ACT.

Per-core output: [1, 512] f32 column sums; host reduces and divides.
"""
import sys

sys.path.insert(0, "/opt/trn_rl_repo")

import numpy as np
import ml_dtypes

H = W = 512
N_CORES = 8
PAIRS_PER_CORE = 4
TOTAL_B = 32
SQ63 = float(np.sqrt(63.0))
import os
PM_SQSUM = int(os.environ.get('PM_SQSUM', '1'))
PM_ABSDIFF = int(os.environ.get('PM_ABSDIFF', '1'))

# (out_row_start, M, in_row_start, K, variant_idx)
ROW_TILES = [
    (0, 127, 0, 128, 0),
    (127, 126, 126, 128, 1),
    (253, 126, 252, 128, 1),
    (379, 126, 378, 128, 1),
    (505, 7, 504, 8, 2),
]
NT = len(ROW_TILES)  # 5
N_ACC_COLS = PAIRS_PER_CORE * NT  # 20


def _build_bands_np():
    """[128, 12*128] f32: blocks (band*3 + variant), bands sv, -sv, dv, 2dv.
    All scaled by sqrt(63) so that q = gx'^2 + gy'^2 + 1 = 63*s + 1."""
    mats = {}
    for v, (K, M) in enumerate([(128, 127), (128, 126), (8, 7)]):
        sv = np.zeros((128, 128), np.float32)
        dv = np.zeros((128, 128), np.float32)
        if v == 0:  # first: m=0 clamps row -1 -> 0
            sv[0, 0], sv[1, 0] = 3.0, 1.0
            dv[0, 0], dv[1, 0] = 1.0, -1.0
            for m in range(1, M):
                sv[m - 1, m], sv[m, m], sv[m + 1, m] = 1.0, 2.0, 1.0
                dv[m - 1, m], dv[m + 1, m] = 1.0, -1.0
        elif v == 1:  # mid
            for m in range(M):
                sv[m, m], sv[m + 1, m], sv[m + 2, m] = 1.0, 2.0, 1.0
                dv[m, m], dv[m + 2, m] = 1.0, -1.0
        else:  # last: m=M-1 (global 511) clamps row 512 -> 511
            for m in range(M - 1):
                sv[m, m], sv[m + 1, m], sv[m + 2, m] = 1.0, 2.0, 1.0
                dv[m, m], dv[m + 2, m] = 1.0, -1.0
            m = M - 1
            sv[m, m], sv[m + 1, m] = 1.0, 3.0
            dv[m, m], dv[m + 1, m] = 1.0, -1.0
        mats[(0, v)] = sv
        mats[(1, v)] = -sv
        mats[(2, v)] = dv
        mats[(3, v)] = 2.0 * dv
    w = np.zeros((128, 12 * 128), np.float32)
    for b in range(4):
        for v in range(3):
            w[:, (b * 3 + v) * 128 : (b * 3 + v + 1) * 128] = SQ63 * mats[(b, v)]
    return w.astype(ml_dtypes.bfloat16)


def _sqsum_uops_2x():
    """2x_1P program for SQSUM: per cycle compute q = a^2 + b^2 + 1 for both
    the lo and hi packed bf16 elements. 8 ALU blocks, chains 0-4."""
    from concourse.dve_uop import (
        UopConfig, InpSel, AluInp, DelayInp, OutSel, OutPath, Trigger, ENABLE,
    )
    from concourse.dve_uop import AluOp as UAlu

    u = UopConfig()
    u.enable_input(InpSel.SRC_0, 1)     # chain0 = a_lo
    u.enable_input(InpSel.SRC_1, 2)     # chain1 = b_lo
    u.enable_input(InpSel.SRC_0_HI, 3)  # chain2 = a_hi
    u.enable_input(InpSel.SRC_1_HI, 4)  # chain3 = b_hi
    u.enable_input(InpSel.ONE_F32, 5)   # chain4 = 1.0
    u.require_inp0 = ENABLE
    u.require_inp1 = ENABLE
    u.trigger = (Trigger.SRC_TENSOR_DONE, Trigger.NONE, Trigger.NONE)
    dp = u.datapath_config
    dp[0].enable_alu(UAlu.MULTIPLY, AluInp.PREV_DELAY_0, AluInp.PREV_DELAY_0)
    dp[0].pass_through_delay(1, 2, 3, 4)
    dp[1].enable_alu(UAlu.MULTIPLY, AluInp.PREV_DELAY_1, AluInp.PREV_DELAY_1)
    dp[1].enable_delay_from_src(DelayInp.PREV_ALU_OUT, 0)  # a_lo^2
    dp[1].pass_through_delay(2, 3, 4)
    dp[2].enable_alu(UAlu.ADD, AluInp.PREV_ALU_OUT, AluInp.PREV_DELAY_0)
    dp[2].pass_through_delay(2, 3, 4)
    dp[3].enable_alu(UAlu.ADD, AluInp.PREV_ALU_OUT, AluInp.PREV_DELAY_4)
    dp[3].pass_through_delay(2, 3, 4)
    dp[4].enable_alu(UAlu.MULTIPLY, AluInp.PREV_DELAY_2, AluInp.PREV_DELAY_2)
    dp[4].enable_delay_from_src(DelayInp.PREV_ALU_OUT, 0)  # q_lo
    dp[4].pass_through_delay(3, 4)
    dp[5].enable_alu(UAlu.MULTIPLY, AluInp.PREV_DELAY_3, AluInp.PREV_DELAY_3)
    dp[5].enable_delay_from_src(DelayInp.PREV_ALU_OUT, 1)  # a_hi^2
    dp[5].pass_through_delay(0, 4)
    dp[6].enable_alu(UAlu.ADD, AluInp.PREV_ALU_OUT, AluInp.PREV_DELAY_1)
    dp[6].pass_through_delay(0, 4)
    dp[7].enable_alu(UAlu.ADD, AluInp.PREV_ALU_OUT, AluInp.PREV_DELAY_4)
    dp[7].pass_through_delay(0)
    u.enable_output(OutSel.DELAY_0, OutPath.WR0_LO)  # q_lo
    u.enable_output(OutSel.ALU_OUT, OutPath.WR0_HI)  # q_hi
    return [u]


def _absdiff2_uops_2x():
    """2x_1P program for plain ABSDIFF (no accumulator): per cycle |a-b| for
    lo and hi packed bf16 elements. hi marches down the ALU bypass chain to
    block 7 (ALU_OUT); lo rides delay chain 0."""
    from concourse.dve_uop import (
        UopConfig, InpSel, AluInp, DelayInp, OutSel, OutPath, Trigger, ENABLE,
    )
    from concourse.dve_uop import AluOp as UAlu

    u = UopConfig()
    u.enable_input(InpSel.SRC_0, 1)     # chain0 = a_lo
    u.enable_input(InpSel.SRC_1, 2)     # chain1 = b_lo
    u.enable_input(InpSel.SRC_0_HI, 3)  # chain2 = a_hi
    u.enable_input(InpSel.SRC_1_HI, 4)  # chain3 = b_hi
    u.require_inp0 = ENABLE
    u.require_inp1 = ENABLE
    u.trigger = (Trigger.SRC_TENSOR_DONE, Trigger.NONE, Trigger.NONE)
    dp = u.datapath_config
    dp[0].enable_alu(UAlu.ABSOLUTE_DIFF, AluInp.PREV_DELAY_0, AluInp.PREV_DELAY_1)
    dp[0].pass_through_delay(2, 3)
    dp[1].enable_alu(UAlu.ABSOLUTE_DIFF, AluInp.PREV_DELAY_2, AluInp.PREV_DELAY_3)
    dp[1].enable_delay_from_src(DelayInp.PREV_ALU_OUT, 0)  # |d|_lo
    for bi in range(2, 8):
        dp[bi].pass_through_alu()
        dp[bi].pass_through_delay(0)
    u.enable_output(OutSel.DELAY_0, OutPath.WR0_LO)  # |d|_lo
    u.enable_output(OutSel.ALU_OUT, OutPath.WR0_HI)  # |d|_hi
    return [u]


def _absdiff_uops_2x():
    """2x_1P program for ABSDIFF_ACC: per cycle |a-b| for lo and hi packed
    elements, elementwise outs on WR0_LO/WR0_HI, accumulator += lo+hi.
    Two uops: [0] = 1-cycle accumulator seed (Zero), [1] = steady state."""
    from concourse.dve_uop import (
        UopConfig, InpSel, AluInp, DelayInp, OutSel, OutPath, Trigger, ENABLE,
    )
    from concourse.dve_uop import AluOp as UAlu

    def lanes(u):
        u.enable_input(InpSel.SRC_0, 1)     # chain0 = a_lo
        u.enable_input(InpSel.SRC_1, 2)     # chain1 = b_lo
        u.enable_input(InpSel.SRC_0_HI, 3)  # chain2 = a_hi
        u.enable_input(InpSel.SRC_1_HI, 4)  # chain3 = b_hi
        u.enable_input(InpSel.ZERO, 5)      # chain4 = 0.0 (accum seed)

    u0 = UopConfig()
    lanes(u0)
    u0.repeat_count = 1
    u0.trigger = (Trigger.COUNT, Trigger.NONE, Trigger.NONE)
    u0.next_uop = (1, 0, 0)
    u0.accum_enabled = ENABLE
    dp = u0.datapath_config
    # carry the ZERO const (chain4) to block 3 and seed the accumulator there
    # (steady state's accumulate-ADD reads CURR_ALU_OUT at block 3).
    for bi in range(3):
        dp[bi].pass_through_alu()
        dp[bi].pass_through_delay(4)
    dp[3].enable_alu(UAlu.BYPASS, AluInp.PREV_DELAY_4, AluInp.PREV_DELAY_4)
    dp[3].alu_out_a_enable = ENABLE
    for bi in range(4, 8):
        dp[bi].pass_through_alu()
        dp[bi].alu_out_a_enable = ENABLE

    u1 = UopConfig()
    lanes(u1)
    u1.require_inp0 = ENABLE
    u1.require_inp1 = ENABLE
    u1.trigger = (Trigger.SRC_TENSOR_DONE, Trigger.NONE, Trigger.NONE)
    u1.accum_enabled = ENABLE
    dp = u1.datapath_config
    dp[0].enable_alu(UAlu.ABSOLUTE_DIFF, AluInp.PREV_DELAY_0, AluInp.PREV_DELAY_1)
    dp[0].pass_through_delay(2, 3)
    dp[1].enable_alu(UAlu.ABSOLUTE_DIFF, AluInp.PREV_DELAY_2, AluInp.PREV_DELAY_3)
    dp[1].enable_delay_from_src(DelayInp.PREV_ALU_OUT, 0)  # |d|_lo
    dp[2].enable_alu(UAlu.ADD, AluInp.PREV_ALU_OUT, AluInp.PREV_DELAY_0)
    dp[2].enable_delay_from_src(DelayInp.PREV_ALU_OUT, 1)  # |d|_hi
    dp[2].pass_through_delay(0)
    dp[3].enable_alu(UAlu.ADD, AluInp.CURR_ALU_OUT, AluInp.PREV_ALU_OUT)
    dp[3].alu_out_a_enable = ENABLE
    dp[3].pass_through_delay(0, 1)
    for bi in range(4, 8):
        dp[bi].pass_through_alu()
        dp[bi].alu_out_a_enable = ENABLE
        dp[bi].pass_through_delay(0, 1)
    u1.enable_output(OutSel.DELAY_0, OutPath.WR0_LO)  # |d|_lo
    u1.enable_output(OutSel.DELAY_1, OutPath.WR0_HI)  # |d|_hi
    return [u0, u1]


def _register_custom_ops():
    """Register the two fused DVE ops into concourse.dve_ops at runtime,
    with hand-authored 2x_1P perf-mode programs."""
    from concourse import dve_ops
    from concourse.dve_spec import (
        Spec, Src0, Src1, One, Zero, Bin, AluOp, sq, lower, _has_src1,
    )
    from concourse.dve_uop import DveOpSpec
    from operator import add as _add
    from dataclasses import dataclass, field as _field

    @dataclass(frozen=True)
    class DveOp2x(dve_ops.DveOp):
        uops_2x_builder: object = None

        def compile(self, ver):
            key = (self.name, ver)
            if (r := dve_ops._COMPILE_CACHE.get(key)) is not None:
                return r
            spec = DveOpSpec(
                name=self.name,
                opcode=dve_ops.get_dve_sub_opcode(self.name),
                uops=lower(self.spec, ver=ver),
                rd1_en=_has_src1(self.spec),
            )
            if ver == "v3" and self.uops_2x_builder is not None:
                spec.uops_2x = self.uops_2x_builder()
                spec.perf_max = 1
            dve_ops._COMPILE_CACHE[key] = spec
            return spec

    def mk(name, spec, subdim=False, uops_2x_builder=None):
        for o in dve_ops.OPS:
            if o.name == name:
                return o
        op = DveOp2x(
            name, spec, subdim=subdim, uops_sha={}, uops_2x_builder=uops_2x_builder
        )
        dve_ops.OPS.append(op)
        dve_ops.CUSTOM_DVE_SPECS[name] = spec
        dve_ops._SUB_OPCODE_FOR_NAME[name] = (
            dve_ops._CUSTOM_DVE_ROW_BASE + len(dve_ops.OPS) - 1
        )
        return op

    sqsum = mk(
        "SQSUM1_HNL",
        Spec(
            body=sq(Src0) + sq(Src1) + One,
            reference=lambda in0, in1, s0, s1, imm2: (
                in0.astype(np.float32) ** 2 + in1.astype(np.float32) ** 2 + 1.0
            ),
        ),
        uops_2x_builder=_sqsum_uops_2x,
    )

    def _absdiff_ref(in0, in1, s0, s1, imm2):
        b = np.abs(in0.astype(np.float32) - in1.astype(np.float32)).astype(np.float32)
        return b, b.reshape(b.shape[0], -1).sum(axis=-1, keepdims=True)

    absdiff = mk(
        "ABSDIFF2_HNL",
        Spec(
            body=Bin(AluOp.ABSOLUTE_DIFF, Src0, Src1),
            reference=lambda in0, in1, s0, s1, imm2: np.abs(
                in0.astype(np.float32) - in1.astype(np.float32)
            ).astype(np.float32),
        ),
        uops_2x_builder=_absdiff2_uops_2x,
    )
    return sqsum, absdiff


def _emit_custom_dve(vec, op, *, out, in0, in1=None, s0=0.0, s1=0.0, imm2=0.0,
                     accum_out=None, perf_max=0):
    """Mirror of bass.Vector._custom_dve that also encodes perf_max (byte-36
    [7:6]) so the engine may select the 2x_1P uop slot."""
    from concourse import bass_isa
    from concourse.bass import dve_ver_for
    from concourse.dve_ops import get_dve_sub_opcode

    bass = vec.bass
    if op.name not in bass.m.ant_custom_dve_ops:
        bass.m.ant_custom_dve_ops = sorted({*bass.m.ant_custom_dve_ops, op.name})
    ver = dve_ver_for(bass.trn_type)
    compiled = op.compile(ver)
    opt = not op.subdim
    in1_elementwise = len(in1.shape) > 2 if in1 is not None else False
    shape = (
        bass_isa.CustomDveShape.STT
        if in1_elementwise
        else bass_isa.CustomDveShape.TTSS
    )
    isa_opcode = bass.isa.Opcode[
        f"NEURON_ISA_TPB_OPCODE_CUSTOM_DVE_ANT_{shape.slot()}"
    ].value
    ins = [vec.lower_ap(in0, for_isa=True, opt=opt)]
    if in1 is not None:
        ins.append(vec.lower_ap(in1, for_isa=True, opt=opt))
    ins += [vec.lower_ap_or_imm(s0, for_isa=True), vec.lower_ap_or_imm(s1, for_isa=True)]
    outs = [vec.lower_ap(out, for_isa=True, opt=opt)]
    if accum_out is not None:
        outs.append(vec.lower_ap(accum_out, for_isa=True))
    return vec.add_instruction(
        bass_isa.InstCustomDveAnt(
            name=bass.get_next_instruction_name(),
            op_name=op.name,
            rd1_en=compiled.rd1_en,
            subdim=0x02 if op.subdim else 0,
            imm2=imm2,
            shape=shape,
            row=get_dve_sub_opcode(op.name),
            isa_opcode=isa_opcode,
            ins=ins,
            outs=outs,
            perf_max=perf_max,
        )
    )


def _kernel_body(tc, gen_d, tgt_d, w_d, acc_d, sqsum_op, absdiff_op):
    from contextlib import ExitStack
    from concourse import mybir
    from concourse.dve_ops import RECIPROCAL_APPROX_FAST, RECIP_APPROX_FAST_CONSTS

    nc = tc.nc
    AF = mybir.ActivationFunctionType
    OP = mybir.AluOpType
    f32 = mybir.dt.float32
    bf16 = mybir.dt.bfloat16

    with ExitStack() as ctx:
        persist = ctx.enter_context(tc.tile_pool(name="persist", bufs=1))
        xp_pool = ctx.enter_context(tc.tile_pool(name="xp", bufs=4))
        ps_pool = ctx.enter_context(tc.tile_pool(name="ps", bufs=3, space="PSUM"))
        acc_ps_pool = ctx.enter_context(tc.tile_pool(name="accps", bufs=1, space="PSUM"))
        gxy_pool = ctx.enter_context(tc.tile_pool(name="gxy", bufs=3))
        q_pool = ctx.enter_context(tc.tile_pool(name="q", bufs=2))
        r_pool = ctx.enter_context(tc.tile_pool(name="r", bufs=3))
        i_pool = ctx.enter_context(tc.tile_pool(name="i", bufs=2))
        n_pool = ctx.enter_context(tc.tile_pool(name="n", bufs=3))
        dd_pool = ctx.enter_context(tc.tile_pool(name="dd", bufs=2))
        xpf_pool = ctx.enter_context(tc.tile_pool(name="xpf", bufs=2))

        wt = persist.tile([128, 12 * 128], bf16)
        nc.sync.dma_start(wt[:], w_d[:])
        ones_t = persist.tile([128, 1], bf16)
        nc.vector.memset(ones_t[:], 1.0)
        bias0 = persist.tile([128, 1], f32)
        nc.vector.memset(bias0[:], 0.0)
        bias_nz = persist.tile([128, 1], f32)
        nc.vector.memset(bias_nz[:], -1.0 / 63.0)
        acc_ps = acc_ps_pool.tile([1, 512], f32)

        def w_sl(band, variant, K, M):
            blk = (band * 3 + variant) * 128
            return wt[0:K, blk : blk + M]

        # Software-pipelined over the 8 images (pair p -> images 2p, 2p+1).
        # stage1(k):  DMA + pads + matmuls + PSUM extraction  (TE + ACT)
        # stage2a(k): q = SQSUM, r = recip                    (DVE)
        # stage2b(k): i'/nz sqrts + nx,ny mults               (ACT + DVE)
        # absdiff(j): after stage2b of image 2j+1             (DVE)
        # Emission order staggers stages so every engine queue always holds
        # independent work (avoids ACT/DVE head-of-line ping-pong).
        N_IMG = 2 * PAIRS_PER_CORE
        state = [dict() for _ in range(N_IMG)]

        def src_of(k):
            return (gen_d if k % 2 == 0 else tgt_d), k // 2

        def stage1_dma(k):
            src, pair = src_of(k)
            xp = xp_pool.tile([128, NT, W + 2], bf16, tag="xp")
            state[k]["xp"] = xp
            if k < 2:
                # pipeline fill: HWDGE (no SWDGE IRAM warmup) f32 loads spread
                # across idle engines' queues + per-rt ACT casts so the first
                # matmuls can start as soon as rt0 lands.
                xpf = xpf_pool.tile([128, NT, W], f32, tag="xpf")
                qeng = [nc.sync, nc.scalar]
                for rt, (r0, M, i0, K, v) in enumerate(ROW_TILES):
                    qeng[(rt + k) % len(qeng)].dma_start(
                        xpf[0:K, rt, :], src[pair, i0 : i0 + K, :]
                    )
                for rt, (r0, M, i0, K, v) in enumerate(ROW_TILES):
                    nc.scalar.copy(xp[:, rt, 1 : W + 1], xpf[:, rt, :])
                    nc.vector.tensor_copy(xp[:, rt, 0:1], xp[:, rt, 1:2])
                    nc.vector.tensor_copy(
                        xp[:, rt, W + 1 : W + 2], xp[:, rt, W : W + 1]
                    )
            else:
                for rt, (r0, M, i0, K, v) in enumerate(ROW_TILES):
                    nc.gpsimd.dma_start(
                        xp[0:K, rt, 1 : W + 1], src[pair, i0 : i0 + K, :]
                    )
                nc.vector.tensor_copy(xp[:, :, 0:1], xp[:, :, 1:2])
                nc.vector.tensor_copy(xp[:, :, W + 1 : W + 2], xp[:, :, W : W + 1])

        def stage1_mm(k):
            gxy = gxy_pool.tile([128, NT, 2 * W], bf16, tag="gxy")
            for rt, (r0, M, i0, K, v) in enumerate(ROW_TILES):
                xr = state[k]["xp"][:, rt, :]
                pgx = ps_pool.tile([128, W], f32, tag="pgx")
                pgy = ps_pool.tile([128, W], f32, tag="pgy")
                dv = w_sl(2, v, K, M)
                nc.tensor.matmul(
                    pgx[0:M, :], w_sl(0, v, K, M), xr[0:K, 0:W], start=True,
                    stop=False,
                )
                nc.tensor.matmul(
                    pgx[0:M, :], w_sl(1, v, K, M), xr[0:K, 2 : W + 2],
                    start=False, stop=True,
                )
                nc.tensor.matmul(
                    pgy[0:M, :], dv, xr[0:K, 0:W], start=True, stop=False
                )
                nc.tensor.matmul(
                    pgy[0:M, :], dv, xr[0:K, 2 : W + 2], start=False, stop=False
                )
                nc.tensor.matmul(
                    pgy[0:M, :], w_sl(3, v, K, M), xr[0:K, 1 : W + 1],
                    start=False, stop=True,
                )
                nc.scalar.copy(gxy[0:M, rt, 0:W], pgx[0:M, :])
                if k < 4:
                    # early images: DVE has slack while ACT is the gate
                    nc.vector.tensor_copy(gxy[0:M, rt, W : 2 * W], pgy[0:M, :])
                else:
                    nc.scalar.copy(gxy[0:M, rt, W : 2 * W], pgy[0:M, :])
            state[k]["gxy"] = gxy

        def stage2a(k):
            gxy = state[k]["gxy"]
            q = q_pool.tile([128, NT, W], bf16, tag="q")
            _emit_custom_dve(
                nc.vector, sqsum_op, out=q[:, :, :],
                in0=gxy[:, :, 0:W], in1=gxy[:, :, W : 2 * W], perf_max=PM_SQSUM,
            )
            r = r_pool.tile([128, NT, W], bf16, tag="r")
            nc.vector._custom_dve(
                RECIPROCAL_APPROX_FAST, out=r[:, :, :], in0=q[:, :, :],
                **RECIP_APPROX_FAST_CONSTS,
            )
            state[k]["r"] = r

        def stage2b(k):
            gxy, r = state[k]["gxy"], state[k]["r"]
            n = n_pool.tile([128, NT, 3 * W], bf16, tag="n")
            ii = i_pool.tile([128, NT, W], bf16, tag="ii")
            nc.scalar.activation(
                ii[:, :, :], r[:, :, :], AF.Sqrt, bias=bias0[:, :], scale=16.0 / 63.0
            )
            nc.scalar.activation(
                n[:, :, 2 * W : 3 * W], r[:, :, :], AF.Sqrt,
                bias=bias_nz[:, :], scale=64.0 / 63.0,
            )
            nc.vector.tensor_tensor(n[:, :, 0:W], gxy[:, :, 0:W], ii[:, :, :], OP.mult)
            nc.vector.tensor_tensor(
                n[:, :, W : 2 * W], gxy[:, :, W : 2 * W], ii[:, :, :], OP.mult
            )
            state[k]["n"] = n

        def absdiff(j, rts=None):
            # |n_gen - n_tgt| (junk rows beyond M are computed but never read
            # by the reduction matmuls). rts selects a row-tile subset so the
            # last pair can interleave absdiff with its reduce matmuls.
            n0, n1 = state[2 * j]["n"], state[2 * j + 1]["n"]
            dd = state[2 * j].get("dd")
            if dd is None:
                dd = dd_pool.tile([128, NT, 3 * W], bf16, tag="dd")
                state[2 * j]["dd"] = dd
            if rts is None:
                _emit_custom_dve(
                    nc.vector, absdiff_op,
                    out=dd[:, :, :], in0=n0[:, :, :], in1=n1[:, :, :],
                    perf_max=PM_ABSDIFF,
                )
            else:
                for rt in rts:
                    _emit_custom_dve(
                        nc.vector, absdiff_op,
                        out=dd[:, rt, :], in0=n0[:, rt, :], in1=n1[:, rt, :],
                        perf_max=PM_ABSDIFF,
                    )

        def reduce_mm(j, first, last, rts=None):
            # sum dd over rows/cols via ones-vector matmuls accumulating into
            # the persistent [1, 512] PSUM tile.
            dd = state[2 * j]["dd"]
            for rt in (range(NT) if rts is None else rts):
                M = ROW_TILES[rt][1]
                for c in range(3):
                    nc.tensor.matmul(
                        acc_ps[0:1, :],
                        ones_t[0:M, 0:1],
                        dd[0:M, rt, c * W : (c + 1) * W],
                        start=(first and rt == 0 and c == 0),
                        stop=(last and rt == NT - 1 and c == 2),
                    )

        JL = PAIRS_PER_CORE - 1  # last pair: interleave per row tile
        for t in range(N_IMG + 3):
            # emission order within a step: DMAs first (latency), then the
            # older images' DVE/ACT chain stages, then this step's matmuls +
            # extraction — so ACT serves i'/nz before the new extractions and
            # DVE's normalize mults don't stall behind them.
            if t < N_IMG:
                stage1_dma(t)
            if 1 <= t <= N_IMG:
                stage2a(t - 1)
            if 2 <= t <= N_IMG + 1:
                k = t - 2
                stage2b(k)
                if k % 2 == 1:
                    j = k // 2
                    if j == JL:
                        for rt in range(NT):
                            absdiff(j, rts=[rt])
                            reduce_mm(j, first=(j == 0), last=True, rts=[rt])
                    else:
                        absdiff(j)
            if 4 <= t and (t - 4) % 2 == 0 and (t - 4) // 2 < JL:
                j = (t - 4) // 2
                reduce_mm(j, first=(j == 0), last=False)
            if t < N_IMG:
                stage1_mm(t)

        acc_sb = persist.tile([1, 512], f32)
        nc.scalar.copy(acc_sb[:], acc_ps[:])
        nc.sync.dma_start(acc_d[:], acc_sb[:])


_CACHE = {}


def _get_module():
    if "nc" not in _CACHE:
        from concourse import bacc, tile, mybir

        sqsum_op, absdiff_op = _register_custom_ops()
        nc = bacc.Bacc(
            "TRN2",
            target_bir_lowering=False,
            debug=False,
            enable_asserts=True,
            num_devices=N_CORES,
        )
        gen_d = nc.dram_tensor(
            "gen", (PAIRS_PER_CORE, H, W), mybir.dt.float32, kind="ExternalInput"
        ).ap()
        tgt_d = nc.dram_tensor(
            "tgt", (PAIRS_PER_CORE, H, W), mybir.dt.float32, kind="ExternalInput"
        ).ap()
        w_d = nc.dram_tensor(
            "w", (128, 12 * 128), mybir.dt.bfloat16, kind="ExternalInput"
        ).ap()
        acc_d = nc.dram_tensor(
            "acc", (1, 512), mybir.dt.float32, kind="ExternalOutput"
        ).ap()
        with tile.TileContext(nc) as tc:
            _kernel_body(tc, gen_d, tgt_d, w_d, acc_d, sqsum_op, absdiff_op)
        nc.compile()
        _CACHE["nc"] = nc
        _CACHE["w"] = _build_bands_np()
    return _CACHE["nc"], _CACHE["w"]


def _run(generated, target, **spmd_kwargs):
    from concourse import bass_utils

    nc, w = _get_module()
    g = np.ascontiguousarray(np.asarray(generated, np.float32).reshape(TOTAL_B, H, W))
    t = np.ascontiguousarray(np.asarray(target, np.float32).reshape(TOTAL_B, H, W))
    in_maps = [
        {
            "gen": g[c * PAIRS_PER_CORE : (c + 1) * PAIRS_PER_CORE],
            "tgt": t[c * PAIRS_PER_CORE : (c + 1) * PAIRS_PER_CORE],
            "w": w,
        }
        for c in range(N_CORES)
    ]
    return bass_utils.run_bass_kernel_spmd(
        nc, in_maps, core_ids=list(range(N_CORES)), **spmd_kwargs
    )


def kernel(generated, target):
    res = _run(generated, target)
    total = 0.0
    for r in res.results:
        total += float(np.asarray(r["acc"], np.float64).sum())
    return np.float32(total / (TOTAL_B * 3 * H * W))
